# revision 13
# baseline (speedup 1.0000x reference)
"""Trainium2 Bass kernel for NinjaTurtleProjLinear: y = x @ (mask*W)^T + b.

Full shapes: x (8192, 2048) f32, weight (8192, 2048) f32, bias (8192,) f32,
sparse_mask (8192, 2048) f32 -> y (8192, 8192) f32.

Strategy (column-parallel over out_features, 8 cores, packed block-sparse):
  - The mask is banded (~1.5% dense). For every [256-out x IN_F] block the
    host gathers the block's nonzero input columns into a dense packed
    contraction axis (exact: dropped columns have mask==0), so each block
    needs only ceil(|support|/128) matmuls instead of 16. One Bass program
    per core; each core's packed x/weight/mask slices are gathered on the
    host (pure layout) and the mask multiply itself runs on DVE.
  - A fully-dense out-row (the gtoken row j=0) would need every input
    column on one core; instead its GEMV is distributed: each packed
    column is owned by exactly one (core, slot), cores accumulate partial
    dot products with a 1-wide stationary matmul, and the host sums the
    partials into y[:, 0].
  - Matmuls are fp16 in / fp32 PSUM accumulate; DVE fuses the bias add
    into the PSUM->SBUF copy; y is stored fp16 on device and upcast on the
    host. Input DMAs ride the SP HWDGE ring, mask/bias/output DMAs the
    ACT ring.
"""
import math
import sys

sys.path.insert(0, "/opt/trn_rl_repo")

import numpy as np

N_TOK = 8192
IN_F = 2048
OUT_F = 8192
N_CORES = 8
OUTF_SH = OUT_F // N_CORES    # 1024
P = 128
TOK_WIN = 512                 # tokens per SBUF window
N_WIN = N_TOK // TOK_WIN      # 16
N_TT = TOK_WIN // P           # 4 token tiles per window
NB = 256                      # out_features per PSUM block
N_NB = OUTF_SH // NB          # 4

_STATE = None


def _analyze(sparse_mask):
    """Per-core packed block schedules + dense-row ownership (exact)."""
    sparse_mask = np.asarray(sparse_mask)
    dense_row = bool(sparse_mask[0].sum() > IN_F // 2)
    pats = []
    for c in range(N_CORES):
        msh = sparse_mask[c * OUTF_SH:(c + 1) * OUTF_SH]
        mb = msh.copy()
        if c == 0 and dense_row:
            mb[0, :] = 0.0
        blocks = []
        for nb in range(N_NB):
            blk = mb[nb * NB:(nb + 1) * NB]
            sup = np.where(blk.any(axis=0))[0].tolist()
            S_b = math.ceil(len(sup) / P)
            rows = sup + [-1] * (S_b * P - len(sup))
            blocks.append(rows)
        pats.append({"blocks": blocks,
                     "zero_row0": c == 0 and dense_row,
                     "extra_rows": []})
    if dense_row:
        owner = set()
        w0_nz = set(np.where(sparse_mask[0] != 0)[0].tolist())
        for c in range(N_CORES):
            own_rows = []
            for rows in pats[c]["blocks"]:
                for col in rows:
                    if col >= 0 and col in w0_nz and col not in owner:
                        owner.add(col)
                        own_rows.append(col)
                    else:
                        own_rows.append(-1)
            pats[c]["own_rows"] = own_rows
        leftover = sorted(w0_nz - owner)
        if leftover:
            S_e = math.ceil(len(leftover) / P)
            extra = leftover + [-1] * (S_e * P - len(leftover))
            pats[0]["extra_rows"] = extra
            pats[0]["own_rows"] = pats[0]["own_rows"] + extra
    else:
        for c in range(N_CORES):
            pats[c]["own_rows"] = []
    return pats, dense_row


def _pack_inputs(pat, c, xt16, weight, bias, sparse_mask):
    """Gather per-core packed operands (host does layout only)."""
    sl = slice(c * OUTF_SH, (c + 1) * OUTF_SH)
    rows_all = [col for rows in pat["blocks"] for col in rows] + pat["extra_rows"]
    idx = np.asarray(rows_all, dtype=np.int64)
    safe = np.clip(idx, 0, None)
    xt_p = xt16[safe].copy()
    xt_p[idx < 0] = 0
    wsh_t = weight[sl].T.astype(np.float16)       # (IN_F, OUTF_SH)
    msh_t = np.asarray(sparse_mask[sl]).T.astype(np.float16).copy()
    if pat["zero_row0"]:
        msh_t[:, 0] = 0
    wt_parts, mt_parts = [], []
    for nb, rows in enumerate(pat["blocks"]):
        bi = np.asarray(rows, dtype=np.int64)
        bs = np.clip(bi, 0, None)
        wt_b = wsh_t[bs, nb * NB:(nb + 1) * NB].copy()
        mt_b = msh_t[bs, nb * NB:(nb + 1) * NB].copy()
        wt_b[bi < 0] = 0
        mt_b[bi < 0] = 0
        wt_parts.append(wt_b)
        mt_parts.append(mt_b)
    ins = {"xt": np.ascontiguousarray(xt_p),
           "wt": np.ascontiguousarray(np.concatenate(wt_parts, axis=0)),
           "mt": np.ascontiguousarray(np.concatenate(mt_parts, axis=0)),
           "b": np.ascontiguousarray(bias[sl])}
    own = pat["own_rows"]
    if own and any(col >= 0 for col in own):
        w0_full = (weight[0] * np.asarray(sparse_mask[0])).astype(np.float16)
        ins["w0"] = np.ascontiguousarray(np.asarray(
            [w0_full[col] if col >= 0 else np.float16(0) for col in own],
            dtype=np.float16))
    return ins


def _build_nc_core(pat):
    import concourse.bass as bass
    import concourse.mybir as mybir
    import concourse.tile as tile
    from concourse import bacc

    f32 = mybir.dt.float32
    f16 = mybir.dt.float16

    blk_S = [len(rows) // P for rows in pat["blocks"]]
    blk_ofs = []
    o = 0
    for s in blk_S:
        blk_ofs.append(o)
        o += s
    S_blocks = o
    S_all = S_blocks + len(pat["extra_rows"]) // P
    own = pat["own_rows"]
    y0_slots = sorted({i // P for i, col in enumerate(own) if col >= 0})
    n0 = len(y0_slots)

    nc = bacc.Bacc(None)
    xt = nc.declare_dram_parameter("xt", [S_all * P, N_TOK], f16, isOutput=False)
    wt = nc.declare_dram_parameter("wt", [S_blocks * P, NB], f16, isOutput=False)
    mt = nc.declare_dram_parameter("mt", [S_blocks * P, NB], f16, isOutput=False)
    b = nc.declare_dram_parameter("b", [OUTF_SH], f32, isOutput=False)
    y = nc.declare_dram_parameter("y", [N_TOK, OUTF_SH], f16, isOutput=True)
    if n0:
        w0 = nc.declare_dram_parameter("w0", [S_all * P], f16, isOutput=False)
        y0p = nc.declare_dram_parameter("y0p", [1, N_TOK], f32, isOutput=True)

    xt_r = xt[:].rearrange("(s p) t -> p s t", p=P)
    wt_r = wt[:].rearrange("(s p) n -> p s n", p=P)
    mt_r = mt[:].rearrange("(s p) n -> p s n", p=P)

    with tile.TileContext(nc) as tc:
        with (
            tc.tile_pool(name="const", bufs=1) as const_pool,
            tc.tile_pool(name="stage", bufs=3) as stage_pool,
            tc.tile_pool(name="xw", bufs=4) as xpool,
            tc.tile_pool(name="out", bufs=4) as opool,
            tc.tile_pool(name="ps", bufs=6, space="PSUM") as pspool,
            tc.tile_pool(name="ps1", bufs=2, space="PSUM") as ps1pool,
        ):
            bias128 = const_pool.tile([P, OUTF_SH], f32)
            b_ap = b[:]
            b_bcast = bass.AP(tensor=b_ap.tensor, offset=b_ap.offset,
                              ap=[[0, P]] + list(b_ap.ap))
            nc.scalar.dma_start(out=bias128[:], in_=b_bcast)

            wm = const_pool.tile([P, S_blocks, NB], f16)
            for s in range(S_blocks):
                wt_s = stage_pool.tile([P, NB], f16, tag="wt")
                mt_s = stage_pool.tile([P, NB], f16, tag="mt")
                nc.sync.dma_start(out=wt_s[:], in_=wt_r[:, s, :])
                nc.scalar.dma_start(out=mt_s[:], in_=mt_r[:, s, :])
                nc.vector.tensor_mul(wm[:, s, :], wt_s[:], mt_s[:])

            if n0:
                w0_sb = const_pool.tile([P, S_all], f16)
                nc.sync.dma_start(
                    out=w0_sb[:], in_=w0[:].rearrange("(s p) -> p s", p=P))
                y0_sb = const_pool.tile([1, N_TOK], f32)

            for w in range(N_WIN):
                xwin = xpool.tile([P, S_all, TOK_WIN], f16)
                nc.sync.dma_start(
                    out=xwin[:], in_=xt_r[:, :, w * TOK_WIN:(w + 1) * TOK_WIN])
                for tt in range(N_TT):
                    t0 = w * TOK_WIN + tt * P
                    out_t = opool.tile([P, OUTF_SH], f16, tag="out_h",
                                       name=f"out_{w}_{tt}")
                    for nb in range(N_NB):
                        S_b = blk_S[nb]
                        if S_b == 0:
                            nc.vector.tensor_copy(
                                out_t[:, nb * NB:(nb + 1) * NB],
                                bias128[:, nb * NB:(nb + 1) * NB])
                            continue
                        ps = pspool.tile([P, NB], f32, tag="ps",
                                         name=f"ps_{w}_{tt}_{nb}")
                        for j in range(S_b):
                            s = blk_ofs[nb] + j
                            nc.tensor.matmul(
                                ps[:],
                                xwin[:, s, tt * P:(tt + 1) * P],
                                wm[:, s, :],
                                start=(j == 0),
                                stop=(j == S_b - 1),
                            )
                        nc.vector.tensor_add(
                            out_t[:, nb * NB:(nb + 1) * NB], ps[:],
                            bias128[:, nb * NB:(nb + 1) * NB])
                    nc.scalar.dma_start(out=y[t0:t0 + P, :], in_=out_t[:])
                if n0:
                    ps0 = ps1pool.tile([1, TOK_WIN], f32, tag="ps0",
                                       name=f"ps0_{w}")
                    for j, s in enumerate(y0_slots):
                        nc.tensor.matmul(
                            ps0[:],
                            w0_sb[:, s:s + 1],
                            xwin[:, s, :],
                            start=(j == 0),
                            stop=(j == n0 - 1),
                        )
                    nc.vector.tensor_copy(
                        y0_sb[:, w * TOK_WIN:(w + 1) * TOK_WIN], ps0[:])
            if n0:
                nc.scalar.dma_start(out=y0p[:], in_=y0_sb[:])
    nc.compile()
    return nc


def _make_core_runner(nc):
    import jax
    import concourse.mybir as mybir
    from concourse import bass2jax

    partition_name = (nc.partition_id_tensor.name
                      if nc.partition_id_tensor else None)
    in_names, out_names, out_avals = [], [], []
    for alloc in nc.m.functions[0].allocations:
        if not isinstance(alloc, mybir.MemoryLocationSet):
            continue
        name = alloc.memorylocations[0].name
        if alloc.kind == "ExternalInput":
            if name != partition_name:
                in_names.append(name)
        elif alloc.kind == "ExternalOutput":
            out_names.append(name)
            out_avals.append(jax.core.ShapedArray(
                tuple(alloc.tensor_shape), mybir.dt.np(alloc.dtype)))
    n_params = len(in_names)
    n_outs = len(out_names)
    all_in_names = list(in_names) + list(out_names)
    if partition_name is not None:
        all_in_names = all_in_names + [partition_name]

    def _body(*args):
        operands = list(args)
        if partition_name is not None:
            operands.append(bass2jax.partition_id_tensor())
        outs = bass2jax._bass_exec_p.bind(
            *operands,
            out_avals=tuple(out_avals),
            in_names=tuple(all_in_names),
            out_names=tuple(out_names),
            lowering_input_output_aliases=(),
            sim_require_finite=True,
            sim_require_nnan=True,
            nc=nc,
        )
        return tuple(outs)

    donate = tuple(range(n_params, n_params + n_outs))
    fn = jax.jit(_body, donate_argnums=donate, keep_unused=True)
    out_shapes = [tuple(a.shape) for a in out_avals]
    out_dtypes = [a.dtype for a in out_avals]
    return fn, in_names, out_names, out_shapes, out_dtypes


def _make_runner(sparse_mask):
    import jax

    pats, dense_row = _analyze(sparse_mask)
    runners = []
    for c in range(N_CORES):
        nc = _build_nc_core(pats[c])
        runners.append(_make_core_runner(nc))
    devices = jax.devices()[:N_CORES]
    return pats, dense_row, runners, devices


def kernel(x, weight, bias, sparse_mask):
    global _STATE
    import jax

    x = np.asarray(x, dtype=np.float32)
    weight = np.asarray(weight, dtype=np.float32)
    bias = np.asarray(bias, dtype=np.float32)
    sparse_mask = np.asarray(sparse_mask, dtype=np.float32)

    mask_key = hash(sparse_mask.tobytes())
    if _STATE is None or _STATE[0] != mask_key:
        _STATE = (mask_key, _make_runner(sparse_mask))
    _, (pats, dense_row, runners, devices) = _STATE

    xt16 = np.ascontiguousarray(x.T.astype(np.float16))
    futures = []
    for c in range(N_CORES):
        fn, in_names, out_names, out_shapes, out_dtypes = runners[c]
        ins = _pack_inputs(pats[c], c, xt16, weight, bias, sparse_mask)
        args = [jax.device_put(ins[n], devices[c]) for n in in_names]
        zeros = [jax.device_put(np.zeros(s, d), devices[c])
                 for s, d in zip(out_shapes, out_dtypes)]
        futures.append(fn(*args, *zeros))

    y_parts = []
    y0_sum = None
    for c in range(N_CORES):
        _, _, out_names, _, _ = runners[c]
        outs = futures[c]
        om = {n: outs[i] for i, n in enumerate(out_names)}
        y_parts.append(np.asarray(om["y"]).astype(np.float32))
        if "y0p" in om:
            p = np.asarray(om["y0p"]).reshape(N_TOK)
            y0_sum = p if y0_sum is None else y0_sum + p
    y_full = np.concatenate(y_parts, axis=1)
    if dense_row and y0_sum is not None:
        y_full[:, 0] = y0_sum + bias[0]
    return y_full


# revision 14
# speedup vs baseline: 1.0554x; 1.0554x over previous
"""Trainium2 Bass kernel for NinjaTurtleProjLinear: y = x @ (mask*W)^T + b.

Full shapes: x (8192, 2048) f32, weight (8192, 2048) f32, bias (8192,) f32,
sparse_mask (8192, 2048) f32 -> y (8192, 8192) f32.

Strategy (column-parallel over out_features, 8 cores, packed block-sparse):
  - The mask is banded (~1.5% dense). For every [256-out x IN_F] block the
    host gathers the block's nonzero input columns into a dense packed
    contraction axis (exact: dropped columns have mask==0), so each block
    needs only ceil(|support|/128) matmuls instead of 16. One Bass program
    per core; each core's packed x/weight/mask slices are gathered on the
    host (pure layout) and the mask multiply itself runs on DVE.
  - A fully-dense out-row (the gtoken row j=0) would need every input
    column on one core; instead its GEMV is distributed: each packed
    column is owned by exactly one (core, slot), cores accumulate partial
    dot products with a 1-wide stationary matmul, and the host sums the
    partials into y[:, 0].
  - Matmuls are fp16 in / fp32 PSUM accumulate; DVE fuses the bias add
    into the PSUM->SBUF copy; y is stored fp16 on device and upcast on the
    host. Input DMAs ride the SP HWDGE ring, mask/bias/output DMAs the
    ACT ring.
"""
import math
import sys

sys.path.insert(0, "/opt/trn_rl_repo")

import numpy as np

N_TOK = 8192
IN_F = 2048
OUT_F = 8192
N_CORES = 8
OUTF_SH = OUT_F // N_CORES    # 1024
P = 128
TOK_WIN = 512                 # tokens per SBUF window
N_WIN = N_TOK // TOK_WIN      # 16
N_TT = TOK_WIN // P           # 4 token tiles per window
NB = 256                      # out_features per PSUM block
N_NB = OUTF_SH // NB          # 4

_STATE = None


def _analyze(sparse_mask):
    """Per-core packed block schedules + dense-row ownership (exact)."""
    sparse_mask = np.asarray(sparse_mask)
    dense_row = bool(sparse_mask[0].sum() > IN_F // 2)
    pats = []
    for c in range(N_CORES):
        msh = sparse_mask[c * OUTF_SH:(c + 1) * OUTF_SH]
        mb = msh.copy()
        if c == 0 and dense_row:
            mb[0, :] = 0.0
        blocks = []
        for nb in range(N_NB):
            blk = mb[nb * NB:(nb + 1) * NB]
            sup = np.where(blk.any(axis=0))[0].tolist()
            sup.append(-2)        # bias pseudo-column: x==1, w==bias, m==1
            S_b = math.ceil(len(sup) / P)
            rows = sup + [-1] * (S_b * P - len(sup))
            blocks.append(rows)
        pats.append({"blocks": blocks,
                     "zero_row0": c == 0 and dense_row,
                     "extra_rows": []})
    if dense_row:
        owner = set()
        w0_nz = set(np.where(sparse_mask[0] != 0)[0].tolist())
        for c in range(N_CORES):
            own_rows = []
            for rows in pats[c]["blocks"]:
                for col in rows:
                    if col >= 0 and col in w0_nz and col not in owner:
                        owner.add(col)
                        own_rows.append(col)
                    else:
                        own_rows.append(-1)
            pats[c]["own_rows"] = own_rows
        leftover = sorted(w0_nz - owner)
        if leftover:
            S_e = math.ceil(len(leftover) / P)
            extra = leftover + [-1] * (S_e * P - len(leftover))
            pats[0]["extra_rows"] = extra
            pats[0]["own_rows"] = pats[0]["own_rows"] + extra
    else:
        for c in range(N_CORES):
            pats[c]["own_rows"] = []
    return pats, dense_row


def _pack_inputs(pat, c, xt16, weight, bias, sparse_mask):
    """Gather per-core packed operands (host does layout only)."""
    sl = slice(c * OUTF_SH, (c + 1) * OUTF_SH)
    rows_all = [col for rows in pat["blocks"] for col in rows] + pat["extra_rows"]
    idx = np.asarray(rows_all, dtype=np.int64)
    safe = np.clip(idx, 0, None)
    xt_p = xt16[safe].copy()
    xt_p[idx < 0] = 0
    xt_p[idx == -2] = 1.0
    wsh_t = weight[sl].T.astype(np.float16)       # (IN_F, OUTF_SH)
    msh_t = np.asarray(sparse_mask[sl]).T.astype(np.float16).copy()
    if pat["zero_row0"]:
        msh_t[:, 0] = 0
    wt_parts, mt_parts = [], []
    for nb, rows in enumerate(pat["blocks"]):
        bi = np.asarray(rows, dtype=np.int64)
        bs = np.clip(bi, 0, None)
        wt_b = wsh_t[bs, nb * NB:(nb + 1) * NB].copy()
        mt_b = msh_t[bs, nb * NB:(nb + 1) * NB].copy()
        wt_b[bi < 0] = 0
        mt_b[bi < 0] = 0
        bsel = bi == -2
        wt_b[bsel] = bias[sl][nb * NB:(nb + 1) * NB].astype(np.float16)
        mt_b[bsel] = 1.0
        wt_parts.append(wt_b)
        mt_parts.append(mt_b)
    ins = {"xt": np.ascontiguousarray(xt_p),
           "wt": np.ascontiguousarray(np.concatenate(wt_parts, axis=0)),
           "mt": np.ascontiguousarray(np.concatenate(mt_parts, axis=0)),
           "b": np.ascontiguousarray(bias[sl])}
    own = pat["own_rows"]
    if own and any(col >= 0 for col in own):
        w0_full = (weight[0] * np.asarray(sparse_mask[0])).astype(np.float16)
        ins["w0"] = np.ascontiguousarray(np.asarray(
            [w0_full[col] if col >= 0 else np.float16(0) for col in own],
            dtype=np.float16))
    return ins


def _build_nc_core(pat):
    import concourse.bass as bass
    import concourse.mybir as mybir
    import concourse.tile as tile
    from concourse import bacc

    f32 = mybir.dt.float32
    f16 = mybir.dt.float16

    blk_S = [len(rows) // P for rows in pat["blocks"]]
    blk_ofs = []
    o = 0
    for s in blk_S:
        blk_ofs.append(o)
        o += s
    S_blocks = o
    S_all = S_blocks + len(pat["extra_rows"]) // P
    own = pat["own_rows"]
    y0_slots = sorted({i // P for i, col in enumerate(own) if col >= 0})
    n0 = len(y0_slots)

    nc = bacc.Bacc(None)
    xt = nc.declare_dram_parameter("xt", [S_all * P, N_TOK], f16, isOutput=False)
    wt = nc.declare_dram_parameter("wt", [S_blocks * P, NB], f16, isOutput=False)
    mt = nc.declare_dram_parameter("mt", [S_blocks * P, NB], f16, isOutput=False)
    b = nc.declare_dram_parameter("b", [OUTF_SH], f32, isOutput=False)
    y = nc.declare_dram_parameter("y", [N_TOK, OUTF_SH], f16, isOutput=True)
    if n0:
        w0 = nc.declare_dram_parameter("w0", [S_all * P], f16, isOutput=False)
        y0p = nc.declare_dram_parameter("y0p", [1, N_TOK], f32, isOutput=True)

    xt_r = xt[:].rearrange("(s p) t -> p s t", p=P)
    wt_r = wt[:].rearrange("(s p) n -> p s n", p=P)
    mt_r = mt[:].rearrange("(s p) n -> p s n", p=P)

    with tile.TileContext(nc) as tc:
        with (
            tc.tile_pool(name="const", bufs=1) as const_pool,
            tc.tile_pool(name="stage", bufs=3) as stage_pool,
            tc.tile_pool(name="xw", bufs=4) as xpool,
            tc.tile_pool(name="out", bufs=4) as opool,
            tc.tile_pool(name="ps", bufs=6, space="PSUM") as pspool,
            tc.tile_pool(name="ps1", bufs=2, space="PSUM") as ps1pool,
        ):
            bias128 = const_pool.tile([P, OUTF_SH], f32)
            b_ap = b[:]
            b_bcast = bass.AP(tensor=b_ap.tensor, offset=b_ap.offset,
                              ap=[[0, P]] + list(b_ap.ap))
            nc.scalar.dma_start(out=bias128[:], in_=b_bcast)

            wm = const_pool.tile([P, S_blocks, NB], f16)
            for s in range(S_blocks):
                wt_s = stage_pool.tile([P, NB], f16, tag="wt")
                mt_s = stage_pool.tile([P, NB], f16, tag="mt")
                nc.sync.dma_start(out=wt_s[:], in_=wt_r[:, s, :])
                nc.scalar.dma_start(out=mt_s[:], in_=mt_r[:, s, :])
                nc.vector.tensor_mul(wm[:, s, :], wt_s[:], mt_s[:])

            if n0:
                w0_sb = const_pool.tile([P, S_all], f16)
                nc.sync.dma_start(
                    out=w0_sb[:], in_=w0[:].rearrange("(s p) -> p s", p=P))
                y0_sb = const_pool.tile([1, N_TOK], f32)

            for w in range(N_WIN):
                xwin = xpool.tile([P, S_all, TOK_WIN], f16)
                nc.sync.dma_start(
                    out=xwin[:], in_=xt_r[:, :, w * TOK_WIN:(w + 1) * TOK_WIN])
                for tt in range(N_TT):
                    t0 = w * TOK_WIN + tt * P
                    out_t = opool.tile([P, OUTF_SH], f16, tag="out_h",
                                       name=f"out_{w}_{tt}")
                    for nb in range(N_NB):
                        S_b = blk_S[nb]
                        if S_b == 0:
                            nc.vector.tensor_copy(
                                out_t[:, nb * NB:(nb + 1) * NB],
                                bias128[:, nb * NB:(nb + 1) * NB])
                            continue
                        ps = pspool.tile([P, NB], f32, tag="ps",
                                         name=f"ps_{w}_{tt}_{nb}")
                        for j in range(S_b):
                            s = blk_ofs[nb] + j
                            nc.tensor.matmul(
                                ps[:],
                                xwin[:, s, tt * P:(tt + 1) * P],
                                wm[:, s, :],
                                start=(j == 0),
                                stop=(j == S_b - 1),
                            )
                        if nb % 2 == 0:
                            nc.vector.tensor_copy(
                                out_t[:, nb * NB:(nb + 1) * NB], ps[:])
                        else:
                            nc.scalar.copy(
                                out_t[:, nb * NB:(nb + 1) * NB], ps[:])
                    nc.scalar.dma_start(out=y[t0:t0 + P, :], in_=out_t[:])
                if n0:
                    ps0 = ps1pool.tile([1, TOK_WIN], f32, tag="ps0",
                                       name=f"ps0_{w}")
                    for j, s in enumerate(y0_slots):
                        nc.tensor.matmul(
                            ps0[:],
                            w0_sb[:, s:s + 1],
                            xwin[:, s, :],
                            start=(j == 0),
                            stop=(j == n0 - 1),
                        )
                    nc.vector.tensor_copy(
                        y0_sb[:, w * TOK_WIN:(w + 1) * TOK_WIN], ps0[:])
            if n0:
                nc.scalar.dma_start(out=y0p[:], in_=y0_sb[:])
    nc.compile()
    return nc


def _make_core_runner(nc):
    import jax
    import concourse.mybir as mybir
    from concourse import bass2jax

    partition_name = (nc.partition_id_tensor.name
                      if nc.partition_id_tensor else None)
    in_names, out_names, out_avals = [], [], []
    for alloc in nc.m.functions[0].allocations:
        if not isinstance(alloc, mybir.MemoryLocationSet):
            continue
        name = alloc.memorylocations[0].name
        if alloc.kind == "ExternalInput":
            if name != partition_name:
                in_names.append(name)
        elif alloc.kind == "ExternalOutput":
            out_names.append(name)
            out_avals.append(jax.core.ShapedArray(
                tuple(alloc.tensor_shape), mybir.dt.np(alloc.dtype)))
    n_params = len(in_names)
    n_outs = len(out_names)
    all_in_names = list(in_names) + list(out_names)
    if partition_name is not None:
        all_in_names = all_in_names + [partition_name]

    def _body(*args):
        operands = list(args)
        if partition_name is not None:
            operands.append(bass2jax.partition_id_tensor())
        outs = bass2jax._bass_exec_p.bind(
            *operands,
            out_avals=tuple(out_avals),
            in_names=tuple(all_in_names),
            out_names=tuple(out_names),
            lowering_input_output_aliases=(),
            sim_require_finite=True,
            sim_require_nnan=True,
            nc=nc,
        )
        return tuple(outs)

    donate = tuple(range(n_params, n_params + n_outs))
    fn = jax.jit(_body, donate_argnums=donate, keep_unused=True)
    out_shapes = [tuple(a.shape) for a in out_avals]
    out_dtypes = [a.dtype for a in out_avals]
    return fn, in_names, out_names, out_shapes, out_dtypes


def _make_runner(sparse_mask):
    import jax

    pats, dense_row = _analyze(sparse_mask)
    runners = []
    for c in range(N_CORES):
        nc = _build_nc_core(pats[c])
        runners.append(_make_core_runner(nc))
    devices = jax.devices()[:N_CORES]
    return pats, dense_row, runners, devices


def kernel(x, weight, bias, sparse_mask):
    global _STATE
    import jax

    x = np.asarray(x, dtype=np.float32)
    weight = np.asarray(weight, dtype=np.float32)
    bias = np.asarray(bias, dtype=np.float32)
    sparse_mask = np.asarray(sparse_mask, dtype=np.float32)

    mask_key = hash(sparse_mask.tobytes())
    if _STATE is None or _STATE[0] != mask_key:
        _STATE = (mask_key, _make_runner(sparse_mask))
    _, (pats, dense_row, runners, devices) = _STATE

    xt16 = np.ascontiguousarray(x.T.astype(np.float16))
    futures = []
    for c in range(N_CORES):
        fn, in_names, out_names, out_shapes, out_dtypes = runners[c]
        ins = _pack_inputs(pats[c], c, xt16, weight, bias, sparse_mask)
        args = [jax.device_put(ins[n], devices[c]) for n in in_names]
        zeros = [jax.device_put(np.zeros(s, d), devices[c])
                 for s, d in zip(out_shapes, out_dtypes)]
        futures.append(fn(*args, *zeros))

    y_parts = []
    y0_sum = None
    for c in range(N_CORES):
        _, _, out_names, _, _ = runners[c]
        outs = futures[c]
        om = {n: outs[i] for i, n in enumerate(out_names)}
        y_parts.append(np.asarray(om["y"]).astype(np.float32))
        if "y0p" in om:
            p = np.asarray(om["y0p"]).reshape(N_TOK)
            y0_sum = p if y0_sum is None else y0_sum + p
    y_full = np.concatenate(y_parts, axis=1)
    if dense_row and y0_sum is not None:
        y_full[:, 0] = y0_sum + bias[0]
    return y_full


# revision 15
# speedup vs baseline: 1.2905x; 1.2228x over previous
"""Trainium2 Bass kernel for NinjaTurtleProjLinear: y = x @ (mask*W)^T + b.

Full shapes: x (8192, 2048) f32, weight (8192, 2048) f32, bias (8192,) f32,
sparse_mask (8192, 2048) f32 -> y (8192, 8192) f32.

Strategy (column-parallel over out_features, 8 cores, packed block-sparse):
  - The mask is banded (~1.5% dense). For every [256-out x IN_F] block the
    host gathers the block's nonzero input columns into a dense packed
    contraction axis (exact: dropped columns have mask==0), so each block
    needs only ceil(|support|/128) matmuls instead of 16. One Bass program
    per core; each core's packed x/weight/mask slices are gathered on the
    host (pure layout) and the mask multiply itself runs on DVE.
  - A fully-dense out-row (the gtoken row j=0) would need every input
    column on one core; instead its GEMV is distributed: each packed
    column is owned by exactly one (core, slot), cores accumulate partial
    dot products with a 1-wide stationary matmul, and the host sums the
    partials into y[:, 0].
  - Matmuls are fp16 in / fp32 PSUM accumulate; DVE fuses the bias add
    into the PSUM->SBUF copy; y is stored fp16 on device and upcast on the
    host. Input DMAs ride the SP HWDGE ring, mask/bias/output DMAs the
    ACT ring.
"""
import math
import sys

sys.path.insert(0, "/opt/trn_rl_repo")

import numpy as np

N_TOK = 8192
IN_F = 2048
OUT_F = 8192
N_CORES = 8
OUTF_SH = OUT_F // N_CORES    # 1024
P = 128
TOK_WIN = 512                 # tokens per SBUF window
N_WIN = N_TOK // TOK_WIN      # 16
N_TT = TOK_WIN // P           # 4 token tiles per window
NB = 256                      # out_features per PSUM block
N_NB = OUTF_SH // NB          # 4

_STATE = None


def _analyze(sparse_mask):
    """Per-core packed block schedules + dense-row ownership (exact)."""
    sparse_mask = np.asarray(sparse_mask)
    dense_row = bool(sparse_mask[0].sum() > IN_F // 2)
    pats = []
    for c in range(N_CORES):
        msh = sparse_mask[c * OUTF_SH:(c + 1) * OUTF_SH]
        mb = msh.copy()
        if c == 0 and dense_row:
            mb[0, :] = 0.0
        sups = []
        for nb in range(N_NB):
            blk = mb[nb * NB:(nb + 1) * NB]
            sups.append(np.where(blk.any(axis=0))[0].tolist())
        rows = sorted(set().union(*[set(s) for s in sups]))
        # insert each block's bias pseudo-column (x==1, w==bias, m==1)
        # adjacent to the block's first support column so it lands inside
        # the block's packed slot span (usually zero extra matmuls)
        for nb in sorted(range(N_NB), key=lambda n: -(min(sups[n]) if sups[n] else 0)):
            lo = min(sups[nb]) if sups[nb] else 0
            at = next((i for i, col in enumerate(rows) if col >= lo), len(rows))
            rows.insert(at, -(2 + nb))
        S_u = math.ceil(len(rows) / P)
        rows = rows + [-1] * (S_u * P - len(rows))
        pos = {col: i for i, col in enumerate(rows)}
        klists = []
        for nb in range(N_NB):
            slots = {pos[col] // P for col in sups[nb]}
            slots.add(pos[-(2 + nb)] // P)
            klists.append(sorted(slots))
        pats.append({"rows": rows, "klists": klists,
                     "zero_row0": c == 0 and dense_row,
                     "extra_rows": []})
    if dense_row:
        owner = set()
        w0_nz = set(np.where(sparse_mask[0] != 0)[0].tolist())
        for c in range(N_CORES):
            own_rows = []
            for col in pats[c]["rows"]:
                if col >= 0 and col in w0_nz and col not in owner:
                    owner.add(col)
                    own_rows.append(col)
                else:
                    own_rows.append(-1)
            pats[c]["own_rows"] = own_rows
        leftover = sorted(w0_nz - owner)
        if leftover:
            S_e = math.ceil(len(leftover) / P)
            extra = leftover + [-1] * (S_e * P - len(leftover))
            pats[0]["extra_rows"] = extra
            pats[0]["own_rows"] = pats[0]["own_rows"] + extra
    else:
        for c in range(N_CORES):
            pats[c]["own_rows"] = []
    return pats, dense_row


def _pack_inputs(pat, c, xt16, weight, bias, sparse_mask):
    """Gather per-core packed operands (host does layout only)."""
    sl = slice(c * OUTF_SH, (c + 1) * OUTF_SH)
    rows_all = list(pat["rows"]) + pat["extra_rows"]
    idx = np.asarray(rows_all, dtype=np.int64)
    safe = np.clip(idx, 0, None)
    xt_p = xt16[safe].copy()
    xt_p[idx < 0] = 0
    xt_p[idx <= -2] = 1.0
    wsh_t = weight[sl].T.astype(np.float16)       # (IN_F, OUTF_SH)
    msh_t = np.asarray(sparse_mask[sl]).T.astype(np.float16).copy()
    if pat["zero_row0"]:
        msh_t[:, 0] = 0
    ui = np.asarray(pat["rows"], dtype=np.int64)
    us = np.clip(ui, 0, None)
    wt_p = wsh_t[us].copy()
    mt_p = msh_t[us].copy()
    wt_p[ui < 0] = 0
    mt_p[ui < 0] = 0
    for nb in range(N_NB):
        bsel = ui == -(2 + nb)
        wt_p[bsel, nb * NB:(nb + 1) * NB] = \
            bias[sl][nb * NB:(nb + 1) * NB].astype(np.float16)
        mt_p[bsel, nb * NB:(nb + 1) * NB] = 1.0
    ins = {"xt": np.ascontiguousarray(xt_p),
           "wt": np.ascontiguousarray(wt_p),
           "mt": np.ascontiguousarray(mt_p),
           "b": np.ascontiguousarray(bias[sl])}
    own = pat["own_rows"]
    if own and any(col >= 0 for col in own):
        w0_full = (weight[0] * np.asarray(sparse_mask[0])).astype(np.float16)
        ins["w0"] = np.ascontiguousarray(np.asarray(
            [w0_full[col] if col >= 0 else np.float16(0) for col in own],
            dtype=np.float16))
    return ins


def _build_nc_core(pat):
    import concourse.bass as bass
    import concourse.mybir as mybir
    import concourse.tile as tile
    from concourse import bacc

    f32 = mybir.dt.float32
    f16 = mybir.dt.float16

    klists = pat["klists"]
    S_blocks = len(pat["rows"]) // P
    S_all = S_blocks + len(pat["extra_rows"]) // P
    own = pat["own_rows"]
    y0_slots = sorted({i // P for i, col in enumerate(own) if col >= 0})
    n0 = len(y0_slots)

    nc = bacc.Bacc(None)
    xt = nc.declare_dram_parameter("xt", [S_all * P, N_TOK], f16, isOutput=False)
    wt = nc.declare_dram_parameter("wt", [S_blocks * P, OUTF_SH], f16, isOutput=False)
    mt = nc.declare_dram_parameter("mt", [S_blocks * P, OUTF_SH], f16, isOutput=False)
    b = nc.declare_dram_parameter("b", [OUTF_SH], f32, isOutput=False)
    y = nc.declare_dram_parameter("y", [N_TOK, OUTF_SH], f16, isOutput=True)
    if n0:
        w0 = nc.declare_dram_parameter("w0", [S_all * P], f16, isOutput=False)
        y0p = nc.declare_dram_parameter("y0p", [1, N_TOK], f32, isOutput=True)

    xt_r = xt[:].rearrange("(s p) t -> p s t", p=P)
    wt_r = wt[:].rearrange("(s p) n -> p s n", p=P)
    mt_r = mt[:].rearrange("(s p) n -> p s n", p=P)

    with tile.TileContext(nc) as tc:
        with (
            tc.tile_pool(name="const", bufs=1) as const_pool,
            tc.tile_pool(name="stage", bufs=3) as stage_pool,
            tc.tile_pool(name="xw", bufs=4) as xpool,
            tc.tile_pool(name="out", bufs=4) as opool,
            tc.tile_pool(name="ps", bufs=6, space="PSUM") as pspool,
            tc.tile_pool(name="ps1", bufs=2, space="PSUM") as ps1pool,
        ):
            bias128 = const_pool.tile([P, OUTF_SH], f32)
            b_ap = b[:]
            b_bcast = bass.AP(tensor=b_ap.tensor, offset=b_ap.offset,
                              ap=[[0, P]] + list(b_ap.ap))
            nc.scalar.dma_start(out=bias128[:], in_=b_bcast)

            wm = const_pool.tile([P, S_blocks, OUTF_SH], f16)
            for s in range(S_blocks):
                wt_s = stage_pool.tile([P, OUTF_SH], f16, tag="wt")
                mt_s = stage_pool.tile([P, OUTF_SH], f16, tag="mt")
                nc.sync.dma_start(out=wt_s[:], in_=wt_r[:, s, :])
                nc.scalar.dma_start(out=mt_s[:], in_=mt_r[:, s, :])
                nc.vector.tensor_mul(wm[:, s, :], wt_s[:], mt_s[:])

            if n0:
                w0_sb = const_pool.tile([P, S_all], f16)
                nc.sync.dma_start(
                    out=w0_sb[:], in_=w0[:].rearrange("(s p) -> p s", p=P))
                y0_sb = const_pool.tile([1, N_TOK], f32)

            for w in range(N_WIN):
                xwin = xpool.tile([P, S_all, TOK_WIN], f16)
                nc.sync.dma_start(
                    out=xwin[:], in_=xt_r[:, :, w * TOK_WIN:(w + 1) * TOK_WIN])
                for tt in range(N_TT):
                    t0 = w * TOK_WIN + tt * P
                    out_t = opool.tile([P, OUTF_SH], f16, tag="out_h",
                                       name=f"out_{w}_{tt}")
                    for nb in range(N_NB):
                        kl = klists[nb]
                        if not kl:
                            nc.vector.tensor_copy(
                                out_t[:, nb * NB:(nb + 1) * NB],
                                bias128[:, nb * NB:(nb + 1) * NB])
                            continue
                        ps = pspool.tile([P, NB], f32, tag="ps",
                                         name=f"ps_{w}_{tt}_{nb}")
                        for j, s in enumerate(kl):
                            nc.tensor.matmul(
                                ps[:],
                                xwin[:, s, tt * P:(tt + 1) * P],
                                wm[:, s, nb * NB:(nb + 1) * NB],
                                start=(j == 0),
                                stop=(j == len(kl) - 1),
                            )
                        if nb % 2 == 0:
                            nc.vector.tensor_copy(
                                out_t[:, nb * NB:(nb + 1) * NB], ps[:])
                        else:
                            nc.scalar.copy(
                                out_t[:, nb * NB:(nb + 1) * NB], ps[:])
                    nc.scalar.dma_start(out=y[t0:t0 + P, :], in_=out_t[:])
                if n0:
                    ps0 = ps1pool.tile([1, TOK_WIN], f32, tag="ps0",
                                       name=f"ps0_{w}")
                    for j, s in enumerate(y0_slots):
                        nc.tensor.matmul(
                            ps0[:],
                            w0_sb[:, s:s + 1],
                            xwin[:, s, :],
                            start=(j == 0),
                            stop=(j == n0 - 1),
                        )
                    nc.vector.tensor_copy(
                        y0_sb[:, w * TOK_WIN:(w + 1) * TOK_WIN], ps0[:])
            if n0:
                nc.scalar.dma_start(out=y0p[:], in_=y0_sb[:])
    nc.compile()
    return nc


def _make_core_runner(nc):
    import jax
    import concourse.mybir as mybir
    from concourse import bass2jax

    partition_name = (nc.partition_id_tensor.name
                      if nc.partition_id_tensor else None)
    in_names, out_names, out_avals = [], [], []
    for alloc in nc.m.functions[0].allocations:
        if not isinstance(alloc, mybir.MemoryLocationSet):
            continue
        name = alloc.memorylocations[0].name
        if alloc.kind == "ExternalInput":
            if name != partition_name:
                in_names.append(name)
        elif alloc.kind == "ExternalOutput":
            out_names.append(name)
            out_avals.append(jax.core.ShapedArray(
                tuple(alloc.tensor_shape), mybir.dt.np(alloc.dtype)))
    n_params = len(in_names)
    n_outs = len(out_names)
    all_in_names = list(in_names) + list(out_names)
    if partition_name is not None:
        all_in_names = all_in_names + [partition_name]

    def _body(*args):
        operands = list(args)
        if partition_name is not None:
            operands.append(bass2jax.partition_id_tensor())
        outs = bass2jax._bass_exec_p.bind(
            *operands,
            out_avals=tuple(out_avals),
            in_names=tuple(all_in_names),
            out_names=tuple(out_names),
            lowering_input_output_aliases=(),
            sim_require_finite=True,
            sim_require_nnan=True,
            nc=nc,
        )
        return tuple(outs)

    donate = tuple(range(n_params, n_params + n_outs))
    fn = jax.jit(_body, donate_argnums=donate, keep_unused=True)
    out_shapes = [tuple(a.shape) for a in out_avals]
    out_dtypes = [a.dtype for a in out_avals]
    return fn, in_names, out_names, out_shapes, out_dtypes


def _make_runner(sparse_mask):
    import jax

    pats, dense_row = _analyze(sparse_mask)
    runners = []
    for c in range(N_CORES):
        nc = _build_nc_core(pats[c])
        runners.append(_make_core_runner(nc))
    devices = jax.devices()[:N_CORES]
    return pats, dense_row, runners, devices


def kernel(x, weight, bias, sparse_mask):
    global _STATE
    import jax

    x = np.asarray(x, dtype=np.float32)
    weight = np.asarray(weight, dtype=np.float32)
    bias = np.asarray(bias, dtype=np.float32)
    sparse_mask = np.asarray(sparse_mask, dtype=np.float32)

    mask_key = hash(sparse_mask.tobytes())
    if _STATE is None or _STATE[0] != mask_key:
        _STATE = (mask_key, _make_runner(sparse_mask))
    _, (pats, dense_row, runners, devices) = _STATE

    xt16 = np.ascontiguousarray(x.T.astype(np.float16))
    futures = []
    for c in range(N_CORES):
        fn, in_names, out_names, out_shapes, out_dtypes = runners[c]
        ins = _pack_inputs(pats[c], c, xt16, weight, bias, sparse_mask)
        args = [jax.device_put(ins[n], devices[c]) for n in in_names]
        zeros = [jax.device_put(np.zeros(s, d), devices[c])
                 for s, d in zip(out_shapes, out_dtypes)]
        futures.append(fn(*args, *zeros))

    y_parts = []
    y0_sum = None
    for c in range(N_CORES):
        _, _, out_names, _, _ = runners[c]
        outs = futures[c]
        om = {n: outs[i] for i, n in enumerate(out_names)}
        y_parts.append(np.asarray(om["y"]).astype(np.float32))
        if "y0p" in om:
            p = np.asarray(om["y0p"]).reshape(N_TOK)
            y0_sum = p if y0_sum is None else y0_sum + p
    y_full = np.concatenate(y_parts, axis=1)
    if dense_row and y0_sum is not None:
        y_full[:, 0] = y0_sum + bias[0]
    return y_full


# revision 16
# speedup vs baseline: 1.3008x; 1.0080x over previous
"""Trainium2 Bass kernel for NinjaTurtleProjLinear: y = x @ (mask*W)^T + b.

Full shapes: x (8192, 2048) f32, weight (8192, 2048) f32, bias (8192,) f32,
sparse_mask (8192, 2048) f32 -> y (8192, 8192) f32.

Strategy (column-parallel over out_features, 8 cores, packed block-sparse):
  - The mask is banded (~1.5% dense). For every [256-out x IN_F] block the
    host gathers the block's nonzero input columns into a dense packed
    contraction axis (exact: dropped columns have mask==0), so each block
    needs only ceil(|support|/128) matmuls instead of 16. One Bass program
    per core; each core's packed x/weight/mask slices are gathered on the
    host (pure layout) and the mask multiply itself runs on DVE.
  - A fully-dense out-row (the gtoken row j=0) would need every input
    column on one core; instead its GEMV is distributed: each packed
    column is owned by exactly one (core, slot), cores accumulate partial
    dot products with a 1-wide stationary matmul, and the host sums the
    partials into y[:, 0].
  - Matmuls are fp16 in / fp32 PSUM accumulate; DVE fuses the bias add
    into the PSUM->SBUF copy; y is stored fp16 on device and upcast on the
    host. Input DMAs ride the SP HWDGE ring, mask/bias/output DMAs the
    ACT ring.
"""
import math
import sys

sys.path.insert(0, "/opt/trn_rl_repo")

import numpy as np

N_TOK = 8192
IN_F = 2048
OUT_F = 8192
N_CORES = 8
OUTF_SH = OUT_F // N_CORES    # 1024
P = 128
TOK_WIN = 1024                # tokens per SBUF window
N_WIN = N_TOK // TOK_WIN      # 8
N_TT = TOK_WIN // P           # 8 token tiles per window
NB = 256                      # out_features per PSUM block
N_NB = OUTF_SH // NB          # 4

_STATE = None


def _analyze(sparse_mask):
    """Per-core packed block schedules + dense-row ownership (exact)."""
    sparse_mask = np.asarray(sparse_mask)
    dense_row = bool(sparse_mask[0].sum() > IN_F // 2)
    pats = []
    for c in range(N_CORES):
        msh = sparse_mask[c * OUTF_SH:(c + 1) * OUTF_SH]
        mb = msh.copy()
        if c == 0 and dense_row:
            mb[0, :] = 0.0
        sups = []
        for nb in range(N_NB):
            blk = mb[nb * NB:(nb + 1) * NB]
            sups.append(np.where(blk.any(axis=0))[0].tolist())
        rows = sorted(set().union(*[set(s) for s in sups]))
        # insert each block's bias pseudo-column (x==1, w==bias, m==1)
        # adjacent to the block's first support column so it lands inside
        # the block's packed slot span (usually zero extra matmuls)
        for nb in sorted(range(N_NB), key=lambda n: -(min(sups[n]) if sups[n] else 0)):
            lo = min(sups[nb]) if sups[nb] else 0
            at = next((i for i, col in enumerate(rows) if col >= lo), len(rows))
            rows.insert(at, -(2 + nb))
        S_u = math.ceil(len(rows) / P)
        rows = rows + [-1] * (S_u * P - len(rows))
        pos = {col: i for i, col in enumerate(rows)}
        klists = []
        for nb in range(N_NB):
            slots = {pos[col] // P for col in sups[nb]}
            slots.add(pos[-(2 + nb)] // P)
            klists.append(sorted(slots))
        pats.append({"rows": rows, "klists": klists,
                     "zero_row0": c == 0 and dense_row,
                     "extra_rows": []})
    if dense_row:
        owner = set()
        w0_nz = set(np.where(sparse_mask[0] != 0)[0].tolist())
        for c in range(N_CORES):
            own_rows = []
            for col in pats[c]["rows"]:
                if col >= 0 and col in w0_nz and col not in owner:
                    owner.add(col)
                    own_rows.append(col)
                else:
                    own_rows.append(-1)
            pats[c]["own_rows"] = own_rows
        leftover = sorted(w0_nz - owner)
        if leftover:
            S_e = math.ceil(len(leftover) / P)
            extra = leftover + [-1] * (S_e * P - len(leftover))
            pats[0]["extra_rows"] = extra
            pats[0]["own_rows"] = pats[0]["own_rows"] + extra
    else:
        for c in range(N_CORES):
            pats[c]["own_rows"] = []
    return pats, dense_row


def _pack_inputs(pat, c, xt16, weight, bias, sparse_mask):
    """Gather per-core packed operands (host does layout only)."""
    sl = slice(c * OUTF_SH, (c + 1) * OUTF_SH)
    rows_all = list(pat["rows"]) + pat["extra_rows"]
    idx = np.asarray(rows_all, dtype=np.int64)
    safe = np.clip(idx, 0, None)
    xt_p = xt16[safe].copy()
    xt_p[idx < 0] = 0
    xt_p[idx <= -2] = 1.0
    wsh_t = weight[sl].T.astype(np.float16)       # (IN_F, OUTF_SH)
    msh_t = np.asarray(sparse_mask[sl]).T.astype(np.float16).copy()
    if pat["zero_row0"]:
        msh_t[:, 0] = 0
    ui = np.asarray(pat["rows"], dtype=np.int64)
    us = np.clip(ui, 0, None)
    wt_p = wsh_t[us].copy()
    mt_p = msh_t[us].copy()
    wt_p[ui < 0] = 0
    mt_p[ui < 0] = 0
    for nb in range(N_NB):
        bsel = ui == -(2 + nb)
        wt_p[bsel, nb * NB:(nb + 1) * NB] = \
            bias[sl][nb * NB:(nb + 1) * NB].astype(np.float16)
        mt_p[bsel, nb * NB:(nb + 1) * NB] = 1.0
    ins = {"xt": np.ascontiguousarray(xt_p),
           "wt": np.ascontiguousarray(wt_p),
           "mt": np.ascontiguousarray(mt_p),
           "b": np.ascontiguousarray(bias[sl])}
    own = pat["own_rows"]
    if own and any(col >= 0 for col in own):
        w0_full = (weight[0] * np.asarray(sparse_mask[0])).astype(np.float16)
        ins["w0"] = np.ascontiguousarray(np.asarray(
            [w0_full[col] if col >= 0 else np.float16(0) for col in own],
            dtype=np.float16))
    return ins


def _build_nc_core(pat):
    import concourse.bass as bass
    import concourse.mybir as mybir
    import concourse.tile as tile
    from concourse import bacc

    f32 = mybir.dt.float32
    f16 = mybir.dt.float16

    klists = pat["klists"]
    S_blocks = len(pat["rows"]) // P
    S_all = S_blocks + len(pat["extra_rows"]) // P
    own = pat["own_rows"]
    y0_slots = sorted({i // P for i, col in enumerate(own) if col >= 0})
    n0 = len(y0_slots)

    nc = bacc.Bacc(None)
    xt = nc.declare_dram_parameter("xt", [S_all * P, N_TOK], f16, isOutput=False)
    wt = nc.declare_dram_parameter("wt", [S_blocks * P, OUTF_SH], f16, isOutput=False)
    mt = nc.declare_dram_parameter("mt", [S_blocks * P, OUTF_SH], f16, isOutput=False)
    b = nc.declare_dram_parameter("b", [OUTF_SH], f32, isOutput=False)
    y = nc.declare_dram_parameter("y", [N_TOK, OUTF_SH], f16, isOutput=True)
    if n0:
        w0 = nc.declare_dram_parameter("w0", [S_all * P], f16, isOutput=False)
        y0p = nc.declare_dram_parameter("y0p", [1, N_TOK], f32, isOutput=True)

    xt_r = xt[:].rearrange("(s p) t -> p s t", p=P)
    wt_r = wt[:].rearrange("(s p) n -> p s n", p=P)
    mt_r = mt[:].rearrange("(s p) n -> p s n", p=P)

    with tile.TileContext(nc) as tc:
        with (
            tc.tile_pool(name="const", bufs=1) as const_pool,
            tc.tile_pool(name="stage", bufs=3) as stage_pool,
            tc.tile_pool(name="xw", bufs=4) as xpool,
            tc.tile_pool(name="out", bufs=4) as opool,
            tc.tile_pool(name="ps", bufs=6, space="PSUM") as pspool,
            tc.tile_pool(name="ps1", bufs=2, space="PSUM") as ps1pool,
        ):
            bias128 = const_pool.tile([P, OUTF_SH], f32)
            b_ap = b[:]
            b_bcast = bass.AP(tensor=b_ap.tensor, offset=b_ap.offset,
                              ap=[[0, P]] + list(b_ap.ap))
            nc.scalar.dma_start(out=bias128[:], in_=b_bcast)

            wm = const_pool.tile([P, S_blocks, OUTF_SH], f16)
            for s in range(S_blocks):
                wt_s = stage_pool.tile([P, OUTF_SH], f16, tag="wt")
                mt_s = stage_pool.tile([P, OUTF_SH], f16, tag="mt")
                nc.sync.dma_start(out=wt_s[:], in_=wt_r[:, s, :])
                nc.scalar.dma_start(out=mt_s[:], in_=mt_r[:, s, :])
                nc.vector.tensor_mul(wm[:, s, :], wt_s[:], mt_s[:])

            if n0:
                w0_sb = const_pool.tile([P, S_all], f16)
                nc.sync.dma_start(
                    out=w0_sb[:], in_=w0[:].rearrange("(s p) -> p s", p=P))
                y0_sb = const_pool.tile([1, N_TOK], f32)

            for w in range(N_WIN):
                xwin = xpool.tile([P, S_all, TOK_WIN], f16)
                nc.sync.dma_start(
                    out=xwin[:], in_=xt_r[:, :, w * TOK_WIN:(w + 1) * TOK_WIN])
                for tt in range(N_TT):
                    t0 = w * TOK_WIN + tt * P
                    out_t = opool.tile([P, OUTF_SH], f16, tag="out_h",
                                       name=f"out_{w}_{tt}")
                    live = [nb for nb in range(N_NB) if klists[nb]]
                    pss = {nb: pspool.tile([P, NB], f32, tag="ps",
                                           name=f"ps_{w}_{tt}_{nb}")
                           for nb in live}
                    if w == 0:
                        # k-outer: each weight slot feeds all live groups as
                        # soon as its DMA + mask-multiply lands, so the PE
                        # pipelines with the weight prologue.
                        for s in range(S_blocks):
                            for nb in live:
                                kl = klists[nb]
                                if s not in kl:
                                    continue
                                nc.tensor.matmul(
                                    pss[nb][:],
                                    xwin[:, s, tt * P:(tt + 1) * P],
                                    wm[:, s, nb * NB:(nb + 1) * NB],
                                    start=(s == kl[0]),
                                    stop=(s == kl[-1]),
                                )
                    else:
                        for nb in live:
                            kl = klists[nb]
                            for j, s in enumerate(kl):
                                nc.tensor.matmul(
                                    pss[nb][:],
                                    xwin[:, s, tt * P:(tt + 1) * P],
                                    wm[:, s, nb * NB:(nb + 1) * NB],
                                    start=(j == 0),
                                    stop=(j == len(kl) - 1),
                                )
                    for nb in range(N_NB):
                        if nb not in pss:
                            nc.vector.tensor_copy(
                                out_t[:, nb * NB:(nb + 1) * NB],
                                bias128[:, nb * NB:(nb + 1) * NB])
                        elif nb % 2 == 0:
                            nc.vector.tensor_copy(
                                out_t[:, nb * NB:(nb + 1) * NB], pss[nb][:])
                        else:
                            nc.scalar.copy(
                                out_t[:, nb * NB:(nb + 1) * NB], pss[nb][:])
                    nc.scalar.dma_start(out=y[t0:t0 + P, :], in_=out_t[:])
                if n0:
                    for hw in range(TOK_WIN // 512):
                        ps0 = ps1pool.tile([1, 512], f32, tag="ps0",
                                           name=f"ps0_{w}_{hw}")
                        for j, s in enumerate(y0_slots):
                            nc.tensor.matmul(
                                ps0[:],
                                w0_sb[:, s:s + 1],
                                xwin[:, s, hw * 512:(hw + 1) * 512],
                                start=(j == 0),
                                stop=(j == n0 - 1),
                            )
                        nc.vector.tensor_copy(
                            y0_sb[:, w * TOK_WIN + hw * 512:
                                  w * TOK_WIN + (hw + 1) * 512], ps0[:])
            if n0:
                nc.scalar.dma_start(out=y0p[:], in_=y0_sb[:])
    nc.compile()
    return nc


def _make_core_runner(nc):
    import jax
    import concourse.mybir as mybir
    from concourse import bass2jax

    partition_name = (nc.partition_id_tensor.name
                      if nc.partition_id_tensor else None)
    in_names, out_names, out_avals = [], [], []
    for alloc in nc.m.functions[0].allocations:
        if not isinstance(alloc, mybir.MemoryLocationSet):
            continue
        name = alloc.memorylocations[0].name
        if alloc.kind == "ExternalInput":
            if name != partition_name:
                in_names.append(name)
        elif alloc.kind == "ExternalOutput":
            out_names.append(name)
            out_avals.append(jax.core.ShapedArray(
                tuple(alloc.tensor_shape), mybir.dt.np(alloc.dtype)))
    n_params = len(in_names)
    n_outs = len(out_names)
    all_in_names = list(in_names) + list(out_names)
    if partition_name is not None:
        all_in_names = all_in_names + [partition_name]

    def _body(*args):
        operands = list(args)
        if partition_name is not None:
            operands.append(bass2jax.partition_id_tensor())
        outs = bass2jax._bass_exec_p.bind(
            *operands,
            out_avals=tuple(out_avals),
            in_names=tuple(all_in_names),
            out_names=tuple(out_names),
            lowering_input_output_aliases=(),
            sim_require_finite=True,
            sim_require_nnan=True,
            nc=nc,
        )
        return tuple(outs)

    donate = tuple(range(n_params, n_params + n_outs))
    fn = jax.jit(_body, donate_argnums=donate, keep_unused=True)
    out_shapes = [tuple(a.shape) for a in out_avals]
    out_dtypes = [a.dtype for a in out_avals]
    return fn, in_names, out_names, out_shapes, out_dtypes


def _make_runner(sparse_mask):
    import jax

    pats, dense_row = _analyze(sparse_mask)
    runners = []
    for c in range(N_CORES):
        nc = _build_nc_core(pats[c])
        runners.append(_make_core_runner(nc))
    devices = jax.devices()[:N_CORES]
    return pats, dense_row, runners, devices


def kernel(x, weight, bias, sparse_mask):
    global _STATE
    import jax

    x = np.asarray(x, dtype=np.float32)
    weight = np.asarray(weight, dtype=np.float32)
    bias = np.asarray(bias, dtype=np.float32)
    sparse_mask = np.asarray(sparse_mask, dtype=np.float32)

    mask_key = hash(sparse_mask.tobytes())
    if _STATE is None or _STATE[0] != mask_key:
        _STATE = (mask_key, _make_runner(sparse_mask))
    _, (pats, dense_row, runners, devices) = _STATE

    xt16 = np.ascontiguousarray(x.T.astype(np.float16))
    futures = []
    for c in range(N_CORES):
        fn, in_names, out_names, out_shapes, out_dtypes = runners[c]
        ins = _pack_inputs(pats[c], c, xt16, weight, bias, sparse_mask)
        args = [jax.device_put(ins[n], devices[c]) for n in in_names]
        zeros = [jax.device_put(np.zeros(s, d), devices[c])
                 for s, d in zip(out_shapes, out_dtypes)]
        futures.append(fn(*args, *zeros))

    y_parts = []
    y0_sum = None
    for c in range(N_CORES):
        _, _, out_names, _, _ = runners[c]
        outs = futures[c]
        om = {n: outs[i] for i, n in enumerate(out_names)}
        y_parts.append(np.asarray(om["y"]).astype(np.float32))
        if "y0p" in om:
            p = np.asarray(om["y0p"]).reshape(N_TOK)
            y0_sum = p if y0_sum is None else y0_sum + p
    y_full = np.concatenate(y_parts, axis=1)
    if dense_row and y0_sum is not None:
        y_full[:, 0] = y0_sum + bias[0]
    return y_full


# revision 17
# speedup vs baseline: 1.3652x; 1.0495x over previous
"""Trainium2 Bass kernel for NinjaTurtleProjLinear: y = x @ (mask*W)^T + b.

Full shapes: x (8192, 2048) f32, weight (8192, 2048) f32, bias (8192,) f32,
sparse_mask (8192, 2048) f32 -> y (8192, 8192) f32.

Strategy (column-parallel over out_features, 8 cores, packed block-sparse):
  - The mask is banded (~1.5% dense). For every [256-out x IN_F] block the
    host gathers the block's nonzero input columns into a dense packed
    contraction axis (exact: dropped columns have mask==0), so each block
    needs only ceil(|support|/128) matmuls instead of 16. One Bass program
    per core; each core's packed x/weight/mask slices are gathered on the
    host (pure layout) and the mask multiply itself runs on DVE.
  - A fully-dense out-row (the gtoken row j=0) would need every input
    column on one core; instead its GEMV is distributed: each packed
    column is owned by exactly one (core, slot), cores accumulate partial
    dot products with a 1-wide stationary matmul, and the host sums the
    partials into y[:, 0].
  - Matmuls are fp16 in / fp32 PSUM accumulate; DVE fuses the bias add
    into the PSUM->SBUF copy; y is stored fp16 on device and upcast on the
    host. Input DMAs ride the SP HWDGE ring, mask/bias/output DMAs the
    ACT ring.
"""
import math
import sys

sys.path.insert(0, "/opt/trn_rl_repo")

import numpy as np

N_TOK = 8192
IN_F = 2048
OUT_F = 8192
N_CORES = 8
OUTF_SH = OUT_F // N_CORES    # 1024
P = 128
TOK_WIN = 1024                # tokens per SBUF window
N_WIN = N_TOK // TOK_WIN      # 8
N_TT = TOK_WIN // P           # 8 token tiles per window
NB = 256                      # out_features per PSUM block
N_NB = OUTF_SH // NB          # 4

_STATE = None


def _analyze(sparse_mask):
    """Per-core packed block schedules + dense-row ownership (exact)."""
    sparse_mask = np.asarray(sparse_mask)
    dense_row = bool(sparse_mask[0].sum() > IN_F // 2)
    pats = []
    for c in range(N_CORES):
        msh = sparse_mask[c * OUTF_SH:(c + 1) * OUTF_SH]
        mb = msh.copy()
        if c == 0 and dense_row:
            mb[0, :] = 0.0
        sups = []
        for nb in range(N_NB):
            blk = mb[nb * NB:(nb + 1) * NB]
            sups.append(np.where(blk.any(axis=0))[0].tolist())
        # order union columns by block-membership interval (keeps each
        # block's columns contiguous), insert each block's bias
        # pseudo-column (x==1, w==bias, m==1) at its first column, then
        # greedily pad (zero rows) before block transitions so block spans
        # start near slot boundaries — minimizes matmuls per block
        memb = {}
        for nb, s in enumerate(sups):
            for col in s:
                memb.setdefault(col, set()).add(nb)
        rows = sorted(memb, key=lambda col: (min(memb[col]), max(memb[col]), col))
        for nb in range(N_NB - 1, -1, -1):
            at = next((i for i, col in enumerate(rows)
                       if col >= 0 and nb in memb[col]), len(rows))
            rows.insert(at, -(2 + nb))
        for nb in range(1, N_NB):
            first_idx = next(
                (i for i, col in enumerate(rows)
                 if (col >= 0 and nb in memb[col] and (nb - 1) not in memb[col])
                 or col == -(2 + nb)), None)
            if first_idx is not None:
                pad = (P - (first_idx % P)) % P
                if pad and pad <= 40:
                    rows = rows[:first_idx] + [-1] * pad + rows[first_idx:]
        S_u = math.ceil(len(rows) / P)
        rows = rows + [-1] * (S_u * P - len(rows))
        pos = {col: i for i, col in enumerate(rows)}
        klists = []
        for nb in range(N_NB):
            slots = {pos[col] // P for col in sups[nb]}
            slots.add(pos[-(2 + nb)] // P)
            klists.append(sorted(slots))
        pats.append({"rows": rows, "klists": klists,
                     "zero_row0": c == 0 and dense_row,
                     "extra_rows": []})
    if dense_row:
        owner = set()
        w0_nz = set(np.where(sparse_mask[0] != 0)[0].tolist())
        for c in range(N_CORES):
            own_rows = []
            for col in pats[c]["rows"]:
                if col >= 0 and col in w0_nz and col not in owner:
                    owner.add(col)
                    own_rows.append(col)
                else:
                    own_rows.append(-1)
            pats[c]["own_rows"] = own_rows
        leftover = sorted(w0_nz - owner)
        if leftover:
            S_e = math.ceil(len(leftover) / P)
            extra = leftover + [-1] * (S_e * P - len(leftover))
            pats[0]["extra_rows"] = extra
            pats[0]["own_rows"] = pats[0]["own_rows"] + extra
    else:
        for c in range(N_CORES):
            pats[c]["own_rows"] = []
    return pats, dense_row


def _pack_inputs(pat, c, xt16, weight, bias, sparse_mask):
    """Gather per-core packed operands (host does layout only)."""
    sl = slice(c * OUTF_SH, (c + 1) * OUTF_SH)
    rows_all = list(pat["rows"]) + pat["extra_rows"]
    idx = np.asarray(rows_all, dtype=np.int64)
    safe = np.clip(idx, 0, None)
    xt_p = xt16[safe].copy()
    xt_p[idx < 0] = 0
    xt_p[idx <= -2] = 1.0
    wsh_t = weight[sl].T.astype(np.float16)       # (IN_F, OUTF_SH)
    msh_t = np.asarray(sparse_mask[sl]).T.astype(np.float16).copy()
    if pat["zero_row0"]:
        msh_t[:, 0] = 0
    ui = np.asarray(pat["rows"], dtype=np.int64)
    us = np.clip(ui, 0, None)
    wt_p = wsh_t[us].copy()
    mt_p = msh_t[us].copy()
    wt_p[ui < 0] = 0
    mt_p[ui < 0] = 0
    for nb in range(N_NB):
        bsel = ui == -(2 + nb)
        wt_p[bsel, nb * NB:(nb + 1) * NB] = \
            bias[sl][nb * NB:(nb + 1) * NB].astype(np.float16)
        mt_p[bsel, nb * NB:(nb + 1) * NB] = 1.0
    ins = {"xt": np.ascontiguousarray(xt_p),
           "wt": np.ascontiguousarray(wt_p),
           "mt": np.ascontiguousarray(mt_p),
           "b": np.ascontiguousarray(bias[sl])}
    own = pat["own_rows"]
    if own and any(col >= 0 for col in own):
        w0_full = (weight[0] * np.asarray(sparse_mask[0])).astype(np.float16)
        ins["w0"] = np.ascontiguousarray(np.asarray(
            [w0_full[col] if col >= 0 else np.float16(0) for col in own],
            dtype=np.float16))
    return ins


def _build_nc_core(pat):
    import concourse.bass as bass
    import concourse.mybir as mybir
    import concourse.tile as tile
    from concourse import bacc

    f32 = mybir.dt.float32
    f16 = mybir.dt.float16

    klists = pat["klists"]
    S_blocks = len(pat["rows"]) // P
    S_all = S_blocks + len(pat["extra_rows"]) // P
    own = pat["own_rows"]
    y0_slots = sorted({i // P for i, col in enumerate(own) if col >= 0})
    n0 = len(y0_slots)

    nc = bacc.Bacc(None)
    xt = nc.declare_dram_parameter("xt", [S_all * P, N_TOK], f16, isOutput=False)
    wt = nc.declare_dram_parameter("wt", [S_blocks * P, OUTF_SH], f16, isOutput=False)
    mt = nc.declare_dram_parameter("mt", [S_blocks * P, OUTF_SH], f16, isOutput=False)
    b = nc.declare_dram_parameter("b", [OUTF_SH], f32, isOutput=False)
    y = nc.declare_dram_parameter("y", [N_TOK, OUTF_SH], f16, isOutput=True)
    if n0:
        w0 = nc.declare_dram_parameter("w0", [S_all * P], f16, isOutput=False)
        y0p = nc.declare_dram_parameter("y0p", [1, N_TOK], f32, isOutput=True)

    xt_r = xt[:].rearrange("(s p) t -> p s t", p=P)
    wt_r = wt[:].rearrange("(s p) n -> p s n", p=P)
    mt_r = mt[:].rearrange("(s p) n -> p s n", p=P)

    with tile.TileContext(nc) as tc:
        with (
            tc.tile_pool(name="const", bufs=1) as const_pool,
            tc.tile_pool(name="stage", bufs=3) as stage_pool,
            tc.tile_pool(name="xw", bufs=4) as xpool,
            tc.tile_pool(name="out", bufs=4) as opool,
            tc.tile_pool(name="ps", bufs=6, space="PSUM") as pspool,
            tc.tile_pool(name="ps1", bufs=2, space="PSUM") as ps1pool,
        ):
            bias128 = const_pool.tile([P, OUTF_SH], f32)
            b_ap = b[:]
            b_bcast = bass.AP(tensor=b_ap.tensor, offset=b_ap.offset,
                              ap=[[0, P]] + list(b_ap.ap))
            nc.scalar.dma_start(out=bias128[:], in_=b_bcast)

            wm = const_pool.tile([P, S_blocks, OUTF_SH], f16)
            for s in range(S_blocks):
                wt_s = stage_pool.tile([P, OUTF_SH], f16, tag="wt")
                mt_s = stage_pool.tile([P, OUTF_SH], f16, tag="mt")
                nc.sync.dma_start(out=wt_s[:], in_=wt_r[:, s, :])
                nc.scalar.dma_start(out=mt_s[:], in_=mt_r[:, s, :])
                nc.vector.tensor_mul(wm[:, s, :], wt_s[:], mt_s[:])

            if n0:
                w0_sb = const_pool.tile([P, S_all], f16)
                nc.sync.dma_start(
                    out=w0_sb[:], in_=w0[:].rearrange("(s p) -> p s", p=P))
                y0_sb = const_pool.tile([1, N_TOK], f32)

            for w in range(N_WIN):
                xwin = xpool.tile([P, S_all, TOK_WIN], f16)
                nc.sync.dma_start(
                    out=xwin[:], in_=xt_r[:, :, w * TOK_WIN:(w + 1) * TOK_WIN])
                for tt in range(N_TT):
                    t0 = w * TOK_WIN + tt * P
                    out_t = opool.tile([P, OUTF_SH], f16, tag="out_h",
                                       name=f"out_{w}_{tt}")
                    live = [nb for nb in range(N_NB) if klists[nb]]
                    pss = {nb: pspool.tile([P, NB], f32, tag="ps",
                                           name=f"ps_{w}_{tt}_{nb}")
                           for nb in live}
                    if w == 0:
                        # k-outer: each weight slot feeds all live groups as
                        # soon as its DMA + mask-multiply lands, so the PE
                        # pipelines with the weight prologue.
                        for s in range(S_blocks):
                            for nb in live:
                                kl = klists[nb]
                                if s not in kl:
                                    continue
                                nc.tensor.matmul(
                                    pss[nb][:],
                                    xwin[:, s, tt * P:(tt + 1) * P],
                                    wm[:, s, nb * NB:(nb + 1) * NB],
                                    start=(s == kl[0]),
                                    stop=(s == kl[-1]),
                                )
                    else:
                        for nb in live:
                            kl = klists[nb]
                            for j, s in enumerate(kl):
                                nc.tensor.matmul(
                                    pss[nb][:],
                                    xwin[:, s, tt * P:(tt + 1) * P],
                                    wm[:, s, nb * NB:(nb + 1) * NB],
                                    start=(j == 0),
                                    stop=(j == len(kl) - 1),
                                )
                    for nb in range(N_NB):
                        if nb not in pss:
                            nc.vector.tensor_copy(
                                out_t[:, nb * NB:(nb + 1) * NB],
                                bias128[:, nb * NB:(nb + 1) * NB])
                        elif nb % 2 == 0:
                            nc.vector.tensor_copy(
                                out_t[:, nb * NB:(nb + 1) * NB], pss[nb][:])
                        else:
                            nc.scalar.copy(
                                out_t[:, nb * NB:(nb + 1) * NB], pss[nb][:])
                    nc.scalar.dma_start(out=y[t0:t0 + P, :], in_=out_t[:])
                if n0:
                    for hw in range(TOK_WIN // 512):
                        ps0 = ps1pool.tile([1, 512], f32, tag="ps0",
                                           name=f"ps0_{w}_{hw}")
                        for j, s in enumerate(y0_slots):
                            nc.tensor.matmul(
                                ps0[:],
                                w0_sb[:, s:s + 1],
                                xwin[:, s, hw * 512:(hw + 1) * 512],
                                start=(j == 0),
                                stop=(j == n0 - 1),
                            )
                        nc.vector.tensor_copy(
                            y0_sb[:, w * TOK_WIN + hw * 512:
                                  w * TOK_WIN + (hw + 1) * 512], ps0[:])
            if n0:
                nc.scalar.dma_start(out=y0p[:], in_=y0_sb[:])
    nc.compile()
    return nc


def _make_core_runner(nc):
    import jax
    import concourse.mybir as mybir
    from concourse import bass2jax

    partition_name = (nc.partition_id_tensor.name
                      if nc.partition_id_tensor else None)
    in_names, out_names, out_avals = [], [], []
    for alloc in nc.m.functions[0].allocations:
        if not isinstance(alloc, mybir.MemoryLocationSet):
            continue
        name = alloc.memorylocations[0].name
        if alloc.kind == "ExternalInput":
            if name != partition_name:
                in_names.append(name)
        elif alloc.kind == "ExternalOutput":
            out_names.append(name)
            out_avals.append(jax.core.ShapedArray(
                tuple(alloc.tensor_shape), mybir.dt.np(alloc.dtype)))
    n_params = len(in_names)
    n_outs = len(out_names)
    all_in_names = list(in_names) + list(out_names)
    if partition_name is not None:
        all_in_names = all_in_names + [partition_name]

    def _body(*args):
        operands = list(args)
        if partition_name is not None:
            operands.append(bass2jax.partition_id_tensor())
        outs = bass2jax._bass_exec_p.bind(
            *operands,
            out_avals=tuple(out_avals),
            in_names=tuple(all_in_names),
            out_names=tuple(out_names),
            lowering_input_output_aliases=(),
            sim_require_finite=True,
            sim_require_nnan=True,
            nc=nc,
        )
        return tuple(outs)

    donate = tuple(range(n_params, n_params + n_outs))
    fn = jax.jit(_body, donate_argnums=donate, keep_unused=True)
    out_shapes = [tuple(a.shape) for a in out_avals]
    out_dtypes = [a.dtype for a in out_avals]
    return fn, in_names, out_names, out_shapes, out_dtypes


def _make_runner(sparse_mask):
    import jax

    pats, dense_row = _analyze(sparse_mask)
    runners = []
    for c in range(N_CORES):
        nc = _build_nc_core(pats[c])
        runners.append(_make_core_runner(nc))
    devices = jax.devices()[:N_CORES]
    return pats, dense_row, runners, devices


def kernel(x, weight, bias, sparse_mask):
    global _STATE
    import jax

    x = np.asarray(x, dtype=np.float32)
    weight = np.asarray(weight, dtype=np.float32)
    bias = np.asarray(bias, dtype=np.float32)
    sparse_mask = np.asarray(sparse_mask, dtype=np.float32)

    mask_key = hash(sparse_mask.tobytes())
    if _STATE is None or _STATE[0] != mask_key:
        _STATE = (mask_key, _make_runner(sparse_mask))
    _, (pats, dense_row, runners, devices) = _STATE

    xt16 = np.ascontiguousarray(x.T.astype(np.float16))
    futures = []
    for c in range(N_CORES):
        fn, in_names, out_names, out_shapes, out_dtypes = runners[c]
        ins = _pack_inputs(pats[c], c, xt16, weight, bias, sparse_mask)
        args = [jax.device_put(ins[n], devices[c]) for n in in_names]
        zeros = [jax.device_put(np.zeros(s, d), devices[c])
                 for s, d in zip(out_shapes, out_dtypes)]
        futures.append(fn(*args, *zeros))

    y_parts = []
    y0_sum = None
    for c in range(N_CORES):
        _, _, out_names, _, _ = runners[c]
        outs = futures[c]
        om = {n: outs[i] for i, n in enumerate(out_names)}
        y_parts.append(np.asarray(om["y"]).astype(np.float32))
        if "y0p" in om:
            p = np.asarray(om["y0p"]).reshape(N_TOK)
            y0_sum = p if y0_sum is None else y0_sum + p
    y_full = np.concatenate(y_parts, axis=1)
    if dense_row and y0_sum is not None:
        y_full[:, 0] = y0_sum + bias[0]
    return y_full
